# revision 1
# baseline (speedup 1.0000x reference)
"""GateLoop fused Bass/Tile kernel for Trainium2, SPMD over 8 NeuronCores.

Problem (B=2, S=4096, D=1024):
    xn = rmsnorm(x) * gamma * sqrt(D)         (sum-of-squares norm)
    q,k,v = xn@wq, xn@wk, xn@wv ; a = sigmoid(xn@wa) ; g = xn@wg
    s_t = a_t * s_{t-1} + (k_t*v_t)           (elementwise linear recurrence)
    out = (q*s * silu(g)) @ wo

Sharding: sequence-parallel. Core c handles batch c//4, tokens
[(c%4)*1024, (c%4+1)*1024). The cross-chunk scan carry is resolved with the
decomposition  s = s_local + cumA * s_in:  each core computes per-chunk
summaries (A_total, s_last), AllGathers them (8KB), combines prefixes
locally, and applies its incoming state as a per-channel scalar.

gamma is folded into the five input-side projection weights on the host.
Matmuls run as fp32r (fp32 with 11-bit mantissa) at full PE rate; weights are
pre-rounded on the host, activations are rounded on-device by the ACT/DVE ops
that produce them.
"""

import numpy as np

import concourse.bacc as bacc
import concourse.tile as tile
from concourse import mybir
from concourse.bass_utils import run_bass_kernel_spmd
from concourse.masks import make_identity

AFT = mybir.ActivationFunctionType
ALU = mybir.AluOpType
F32 = mybir.dt.float32
F32R = mybir.dt.float32r
F16 = mybir.dt.float16

B, S, D = 2, 4096, 1024
NCORE = 8
GROUPS = 2              # batch groups of 4 cores
CPG = NCORE // GROUPS   # chunks (cores) per group
CHUNK = (B * S) // NCORE  # 1024 tokens per core
P = 128
NPT = D // P            # 8 channel ptiles
NM = CHUNK // P         # 8 token tiles
H = 512                 # psum half width (fp32 bank)
EPS = 1e-5
YSC = 4096.0            # fp16 range guard: y is carried as y/YSC

_CACHE = {}


def _round_fp32r(a: np.ndarray) -> np.ndarray:
    """RNE-round fp32 to the fp32r (1+8+11-bit) format walrus expects."""
    u = np.ascontiguousarray(a, dtype=np.float32).view(np.uint32).copy()
    u += 0x7FF + ((u >> 12) & 1)
    u &= np.uint32(0xFFFFF000)
    return u.view(np.float32)


def _build(no_collective=False):
    nc = bacc.Bacc("TRN2", target_bir_lowering=False, debug=False,
                   num_devices=NCORE)
    x_in = nc.dram_tensor("x", [CHUNK, D], F32, kind="ExternalInput")
    w_in = {
        n: nc.dram_tensor(n, [NPT, P, D], F16, kind="ExternalInput")
        for n in ("wg", "wq", "wk", "wv", "wa")
    }
    w_in["wo"] = nc.dram_tensor("wo", [2, NPT, P, H], F16,
                                kind="ExternalInput")
    mask_in = nc.dram_tensor("mask", [P, NPT * NCORE], F32,
                             kind="ExternalInput")
    out_t = nc.dram_tensor("out", [CHUNK, D], F32, kind="ExternalOutput")

    with tile.TileContext(nc) as tc:
        with (
            tc.tile_pool(name="const", bufs=1) as const,
            tc.tile_pool(name="xny", bufs=NPT) as xny,
            tc.tile_pool(name="persist", bufs=1) as persist,
            tc.tile_pool(name="scr", bufs=2) as scr,
            tc.tile_pool(name="tiny", bufs=4) as tiny,
            tc.tile_pool(name="wpool", bufs=3) as wpool,
            tc.tile_pool(name="wopool", bufs=8) as wopool,
            tc.tile_pool(name="small", bufs=1) as small,
            tc.tile_pool(name="dram", bufs=1, space="DRAM") as dram,
        ):
            ident = const.tile([P, P], F32)
            make_identity(nc, ident)
            epsb = const.tile([P, 1], F32)
            nc.vector.memset(epsb, EPS / D)

            xnT = [xny.tile([P, CHUNK], F16, tag="xny", name=f"xnT{d}")
                   for d in range(NPT)]

            # ---- Phase A1: rmsnorm + transpose to [channel, token] ----
            with tc.tile_pool(name="pst", bufs=4, space="PSUM") as pstp:
                for m in range(NM):
                    xm = scr.tile([P, D], F32, tag="x", name=f"x{m}")
                    nc.sync.dma_start(out=xm[:], in_=x_in[m * P:(m + 1) * P, :])
                    xn = scr.tile([P, D], F32, tag="xn", name=f"xn{m}")
                    ss = tiny.tile([P, 1], F32, tag="ss", name=f"ss{m}")
                    nc.vector.tensor_mul(xn[:], xm[:], xm[:])
                    nc.vector.tensor_reduce(ss[:], xn[:],
                                            axis=mybir.AxisListType.X,
                                            op=ALU.add)
                    sd = tiny.tile([P, 1], F32, tag="sd", name=f"sd{m}")
                    nc.scalar.activation(sd[:], ss[:], AFT.Sqrt,
                                         bias=epsb[:], scale=1.0 / D)
                    inv = tiny.tile([P, 1], F32, tag="inv", name=f"inv{m}")
                    nc.vector.reciprocal(inv[:], sd[:])
                    nc.scalar.activation(xn[:], xm[:], AFT.Copy, scale=inv[:])
                    for d in range(NPT):
                        pst = pstp.tile([P, P], F32, tag="pst",
                                        name=f"pst{m}_{d}")
                        nc.tensor.transpose(pst[:], xn[:, d * P:(d + 1) * P],
                                            ident[:])
                        nc.scalar.activation(xnT[d][:, m * P:(m + 1) * P],
                                             pst[:], AFT.Copy)

            a_t = [persist.tile([P, CHUNK], F32, tag=f"a{p}", name=f"a{p}")
                   for p in range(NPT)]
            kv_t = [persist.tile([P, CHUNK], F32, tag=f"kv{p}", name=f"kv{p}")
                    for p in range(NPT)]
            qg_t = [persist.tile([P, CHUNK], F32, tag=f"qg{p}", name=f"qg{p}")
                    for p in range(NPT)]
            summ_h = [small.tile([P, NPT], F32, name=f"summ{i}")
                      for i in range(2)]

            # ---- Phase A2: five projections + gating + local scans ----
            with tc.tile_pool(name="psp", bufs=4, space="PSUM") as psp:
                for p in range(NPT):
                    halves = {}
                    qg_p = scr.tile([P, CHUNK], F32, tag="qg",
                                    name=f"qg{p}")
                    for wname, key in (("wg", "g"), ("wq", "q"), ("wk", "k"),
                                       ("wv", "v"), ("wa", "a")):
                        pts = []
                        for h in range(2):
                            pts.append(psp.tile([P, H], F32, tag="psp",
                                                name=f"ps_{key}{p}h{h}"))
                        wt = wpool.tile([P, D], F16, tag="w",
                                        name=f"w_{key}{p}")
                        nc.sync.dma_start(out=wt[:], in_=w_in[wname][p])
                        for k in range(NPT):
                            for h in range(2):
                                nc.tensor.matmul(
                                    pts[h][:],
                                    lhsT=wt[:, k * P:(k + 1) * P],
                                    rhs=xnT[k][:, h * H:(h + 1) * H],
                                    start=(k == 0), stop=(k == NPT - 1))
                        halves[key] = pts

                        # evictions, fused per projection
                        for h in range(2):
                            hs = slice(h * H, (h + 1) * H)
                            if key == "g":
                                gs = scr.tile([P, H], F32, tag="gs",
                                              name=f"gs{p}h{h}")
                                nc.scalar.activation(gs[:], pts[h][:],
                                                     AFT.Silu)
                                halves.setdefault("gs", []).append(gs)
                            elif key == "q":
                                nc.vector.scalar_tensor_tensor(
                                    out=qg_p[:, hs], in0=pts[h][:],
                                    scalar=1.0 / YSC,
                                    in1=halves["gs"][h][:],
                                    op0=ALU.mult, op1=ALU.mult)
                            elif key == "k":
                                ks = scr.tile([P, H], F32, tag="ks",
                                              name=f"ks{p}h{h}")
                                nc.scalar.activation(ks[:], pts[h][:],
                                                     AFT.Copy)
                                halves.setdefault("ks", []).append(ks)
                            elif key == "v":
                                nc.vector.tensor_mul(
                                    kv_t[p][:, hs], pts[h][:],
                                    halves["ks"][h][:])
                            elif key == "a":
                                nc.scalar.activation(a_t[p][:, hs],
                                                     pts[h][:], AFT.Sigmoid)

                    # local scans for this channel ptile
                    sl = scr.tile([P, CHUNK], F32, tag="sl", name=f"sl{p}", bufs=1)
                    nc.vector.tensor_tensor_scan(
                        sl[:], a_t[p][:], kv_t[p][:], 0.0,
                        op0=ALU.mult, op1=ALU.add)
                    ca = scr.tile([P, CHUNK], F32, tag="ca", name=f"ca{p}", bufs=1)
                    nc.vector.tensor_tensor_scan(
                        ca[:], a_t[p][:], a_t[p][:], 1.0,
                        op0=ALU.mult, op1=ALU.bypass)
                    sh, sc = divmod(p, NPT // 2)
                    nc.vector.tensor_copy(summ_h[sh][:, sc:sc + 1],
                                          ca[:, CHUNK - 1:CHUNK])
                    nc.vector.tensor_copy(
                        summ_h[sh][:, NPT // 2 + sc:NPT // 2 + sc + 1],
                        sl[:, CHUNK - 1:CHUNK])
                    # P = qg * s_local  (overwrites kv);  C = qg * cumA
                    # (overwrites a)
                    nc.vector.tensor_mul(kv_t[p][:], qg_p[:], sl[:])
                    nc.vector.tensor_mul(a_t[p][:], qg_p[:], ca[:])

            # ---- summary exchange (two pipelined halves) ----
            HP = NPT // 2
            gath = small.tile([P, NCORE * 2 * NPT], F32)
            sin = small.tile([P, NPT], F32)
            if no_collective:
                nc.vector.memset(gath[:], 0.0)
                nc.vector.memset(sin[:], 0.0)
            else:
                maskt = small.tile([P, NPT, NCORE], F32)
                nc.sync.dma_start(
                    out=maskt[:],
                    in_=mask_in.rearrange("p (a b) -> p a b", a=NPT))
                for half in range(2):
                    cc_in = dram.tile([P, NPT], F32, name=f"cc_in{half}")
                    cc_out = dram.tile([NCORE, P, NPT], F32,
                                       addr_space="Shared",
                                       name=f"cc_out{half}")
                    nc.sync.dma_start(out=cc_in[:], in_=summ_h[half][:])
                    nc.gpsimd.collective_compute(
                        "AllGather", ALU.bypass,
                        replica_groups=[list(range(NCORE))],
                        ins=[cc_in[:]], outs=[cc_out[:]])
                    for c in range(NCORE):
                        nc.sync.dma_start(
                            out=gath[:, c * 2 * NPT + half * NPT:
                                     c * 2 * NPT + (half + 1) * NPT],
                            in_=cc_out[c])

                    def A_of(j, h=half):
                        base = j * 2 * NPT + h * NPT
                        return gath[:, base: base + HP]

                    def s_of(j, h=half):
                        base = j * 2 * NPT + h * NPT + HP
                        return gath[:, base: base + HP]

                    # prefix-combine carries for this half's ptiles
                    cand = small.tile([P, HP, NCORE], F32,
                                      name=f"cand{half}")
                    nc.vector.memset(cand[:], 0.0)
                    u = small.tile([P, HP], F32, name=f"u{half}")
                    tmp = small.tile([P, HP], F32, name=f"tmp{half}")
                    for g in range(GROUPS):
                        base = g * CPG
                        nc.vector.tensor_copy(u[:], s_of(base))
                        nc.vector.tensor_copy(cand[:, :, base + 1], u[:])
                        for jj in range(2, CPG):
                            nc.vector.tensor_mul(tmp[:],
                                                 A_of(base + jj - 1), u[:])
                            nc.vector.tensor_add(u[:], tmp[:],
                                                 s_of(base + jj - 1))
                            nc.vector.tensor_copy(cand[:, :, base + jj],
                                                  u[:])
                    masked = small.tile([P, HP, NCORE], F32,
                                        name=f"masked{half}")
                    nc.vector.tensor_mul(
                        masked[:], cand[:],
                        maskt[:, half * HP:(half + 1) * HP, :])
                    nc.vector.tensor_reduce(
                        sin[:, half * HP:(half + 1) * HP], masked[:],
                        axis=mybir.AxisListType.X, op=ALU.add)

            # ---- Phase B: apply carry, output projection ----
            y_t = [xny.tile([P, CHUNK], F16, tag="xny", name=f"y{p}")
                   for p in range(NPT)]
            for p in range(NPT):
                nc.vector.scalar_tensor_tensor(
                    out=y_t[p][:], in0=a_t[p][:], scalar=sin[:, p:p + 1],
                    in1=kv_t[p][:], op0=ALU.mult, op1=ALU.add)

            with tc.tile_pool(name="pso", bufs=4, space="PSUM") as pso:
                for h in range(2):
                    woh = []
                    for k in range(NPT):
                        wk_ = wopool.tile([P, H], F16, tag="woh",
                                          name=f"wo{h}k{k}")
                        nc.sync.dma_start(out=wk_[:], in_=w_in["wo"][h, k])
                        woh.append(wk_)
                    for m in range(NM):
                        po = pso.tile([P, H], F32, tag="pso",
                                      name=f"po{h}m{m}")
                        for k in range(NPT):
                            nc.tensor.matmul(
                                po[:], lhsT=y_t[k][:, m * P:(m + 1) * P],
                                rhs=woh[k][:],
                                start=(k == 0), stop=(k == NPT - 1))
                        ostg = scr.tile([P, H], F32, tag="ostg",
                                        name=f"ostg{h}m{m}", bufs=3)
                        nc.scalar.activation(ostg[:], po[:], AFT.Copy,
                                             scale=YSC)
                        nc.sync.dma_start(
                            out=out_t[m * P:(m + 1) * P, h * H:(h + 1) * H],
                            in_=ostg[:])

    nc.compile()
    return nc


def _get_nc():
    if "nc" not in _CACHE:
        _CACHE["nc"] = _build()
    return _CACHE["nc"]


def _blk_proj(w):
    # [din, dout] -> [p, r, (k, c)]: per dout-ptile slab, contiguous
    return np.ascontiguousarray(
        w.reshape(NPT, P, NPT, P).transpose(2, 1, 0, 3).reshape(NPT, P, D)
        .astype(np.float16))


def _blk_out(w):
    # [din, dout] -> [h, k, r, c]
    return np.ascontiguousarray(
        w.reshape(NPT, P, 2, H).transpose(2, 0, 1, 3).astype(np.float16))


def _make_in_maps(x, gamma, wq, wk, wv, wa, wg, wo):
    w_eff = {
        "wq": _blk_proj(gamma[:, None] * wq),
        "wk": _blk_proj(gamma[:, None] * wk),
        "wv": _blk_proj(gamma[:, None] * wv),
        "wa": _blk_proj(gamma[:, None] * wa),
        "wg": _blk_proj(gamma[:, None] * wg),
        "wo": _blk_out(wo),
    }
    in_maps = []
    for c in range(NCORE):
        b, ch = divmod(c, CPG)
        mask = np.zeros((P, NPT, NCORE), dtype=np.float32)
        mask[:, :, c] = 1.0
        in_maps.append({
            "x": np.ascontiguousarray(
                x[b, ch * CHUNK:(ch + 1) * CHUNK, :], dtype=np.float32),
            "mask": mask.reshape(P, NPT * NCORE),
            **w_eff,
        })
    return in_maps


def run_device(in_maps, trace=False, **kw):
    return run_bass_kernel_spmd(_get_nc(), in_maps, list(range(NCORE)),
                                trace=trace, **kw)


def _assemble(results):
    out = np.empty((B, S, D), dtype=np.float32)
    for c in range(NCORE):
        b, ch = divmod(c, CPG)
        out[b, ch * CHUNK:(ch + 1) * CHUNK, :] = results[c]["out"]
    return out


def kernel(x, gamma, wq, wk, wv, wa, wg, wo):
    in_maps = _make_in_maps(np.asarray(x), np.asarray(gamma), np.asarray(wq),
                            np.asarray(wk), np.asarray(wv), np.asarray(wa),
                            np.asarray(wg), np.asarray(wo))
    res = run_device(in_maps)
    return _assemble(res.results)



# revision 8
# speedup vs baseline: 1.3171x; 1.3171x over previous
"""GateLoop fused Bass/Tile kernel for Trainium2, SPMD over 8 NeuronCores.

Problem (B=2, S=4096, D=1024):
    xn = rmsnorm(x) * gamma * sqrt(D)         (sum-of-squares norm)
    q,k,v = xn@wq, xn@wk, xn@wv ; a = sigmoid(xn@wa) ; g = xn@wg
    s_t = a_t * s_{t-1} + (k_t*v_t)           (elementwise linear recurrence)
    out = (q*s * silu(g)) @ wo
a
Sharding: sequence-parallel. Core c handles batch c//4, tokens
[(c%4)*1024, (c%4+1)*1024). The cross-chunk scan carry is resolved with the
decomposition  s = s_local + cumA * s_in:  each core computes per-chunk
summaries (A_total, s_last), AllGathers them (8KB), combines prefixes
locally, and applies its incoming state as a per-channel scalar.

Schedule: the AllGather's completion is gated by the slowest core's arrival
(launch skew across the 8 PJRT devices is ~90us), so the kernel is ordered
to bank carry-independent work behind the collective: (k,v,a) projections +
local scans first -> single AllGather of both summary halves -> (g,q)
projections + wo prefetch while the gather is in flight -> only the carry
apply + output projection remain on the dependent tail.

gamma is folded into the five input-side projection weights on the host.
Matmuls run in fp16 at full (double-pumped) PE rate.
"""

import numpy as np

import concourse.bacc as bacc
import concourse.tile as tile
from concourse import mybir
from concourse.bass_utils import run_bass_kernel_spmd
from concourse.masks import make_identity

AFT = mybir.ActivationFunctionType
ALU = mybir.AluOpType
F32 = mybir.dt.float32
F16 = mybir.dt.float16

B, S, D = 2, 4096, 1024
NCORE = 8
GROUPS = 2              # batch groups of 4 cores
CPG = NCORE // GROUPS   # chunks (cores) per group
CHUNK = (B * S) // NCORE  # 1024 tokens per core
P = 128
NPT = D // P            # 8 channel ptiles
NM = CHUNK // P         # 8 token tiles
H = 512                 # psum half width (fp32 bank)
EPS = 1e-5
YSC = 4096.0            # fp16 range guard: y is carried as y/YSC
SW = 2 * NPT            # summary width: [A_total | s_last] per ptile

_CACHE = {}


def _build():
    nc = bacc.Bacc("TRN2", target_bir_lowering=False, debug=False,
                   num_devices=NCORE)
    x_in = nc.dram_tensor("x", [CHUNK, D], F32, kind="ExternalInput")
    w_in = {
        n: nc.dram_tensor(n, [NPT, P, D], F16, kind="ExternalInput")
        for n in ("wg", "wq", "wk", "wv", "wa")
    }
    w_in["wo"] = nc.dram_tensor("wo", [2, NPT, P, H], F16,
                                kind="ExternalInput")
    mask_in = nc.dram_tensor("mask", [P, NPT * NCORE], F32,
                             kind="ExternalInput")
    out_t = nc.dram_tensor("out", [CHUNK, D], F32, kind="ExternalOutput")

    with tc_ctx(nc) as tc:
        with (
            tc.tile_pool(name="const", bufs=1) as const,
            tc.tile_pool(name="xny", bufs=NPT) as xny,
            tc.tile_pool(name="persist", bufs=1) as persist,
            tc.tile_pool(name="scr", bufs=2) as scr,
            tc.tile_pool(name="tiny", bufs=4) as tiny,
            tc.tile_pool(name="wpool", bufs=3) as wpool,
            tc.tile_pool(name="wopool", bufs=16) as wopool,
            tc.tile_pool(name="small", bufs=1) as small,
            tc.tile_pool(name="dram", bufs=1, space="DRAM") as dram,
        ):
            ident = const.tile([P, P], F32)
            make_identity(nc, ident)
            epsb = const.tile([P, 1], F32)
            nc.vector.memset(epsb, EPS / D)
            maskt = small.tile([P, NPT, NCORE], F32)
            nc.sync.dma_start(
                out=maskt[:],
                in_=mask_in.rearrange("p (a b) -> p a b", a=NPT))

            xnT = [xny.tile([P, CHUNK], F16, tag="xny", name=f"xnT{d}")
                   for d in range(NPT)]

            # ---- Phase A1: rmsnorm + transpose to [channel, token] ----
            # ACT does only the Sqrt; everything else lives on DVE/Pool so
            # the activation table is not thrashed.
            with tc.tile_pool(name="pst", bufs=4, space="PSUM") as pstp:
                for m in range(NM):
                    xm = scr.tile([P, D], F32, tag="x", name=f"x{m}")
                    nc.sync.dma_start(out=xm[:], in_=x_in[m * P:(m + 1) * P, :])
                    xn = scr.tile([P, D], F32, tag="xn", name=f"xn{m}")
                    ss = tiny.tile([P, 1], F32, tag="ss", name=f"ss{m}")
                    nc.vector.tensor_mul(xn[:], xm[:], xm[:])
                    nc.vector.tensor_reduce(ss[:], xn[:],
                                            axis=mybir.AxisListType.X,
                                            op=ALU.add)
                    sd = tiny.tile([P, 1], F32, tag="sd", name=f"sd{m}")
                    nc.scalar.activation(sd[:], ss[:], AFT.Sqrt,
                                         bias=epsb[:], scale=1.0 / D)
                    inv = tiny.tile([P, 1], F32, tag="inv", name=f"inv{m}")
                    nc.vector.reciprocal(inv[:], sd[:])
                    nc.scalar.activation(xn[:], xm[:], AFT.Copy, scale=inv[:])
                    for d in range(NPT):
                        pst = pstp.tile([P, P], F32, tag="pst",
                                        name=f"pst{m}_{d}")
                        nc.tensor.transpose(pst[:], xn[:, d * P:(d + 1) * P],
                                            ident[:])
                        nc.scalar.activation(xnT[d][:, m * P:(m + 1) * P],
                                             pst[:], AFT.Copy)

            a_t = [persist.tile([P, CHUNK], F32, tag=f"a{p}", name=f"a{p}")
                   for p in range(NPT)]
            kv_t = [persist.tile([P, CHUNK], F32, tag=f"kv{p}", name=f"kv{p}")
                    for p in range(NPT)]
            sl_t = [persist.tile([P, CHUNK], F32, tag=f"sl{p}", name=f"sl{p}")
                    for p in range(NPT)]
            ca_t = [persist.tile([P, CHUNK], F32, tag=f"ca{p}", name=f"ca{p}")
                    for p in range(NPT)]
            summ = small.tile([P, SW], F32)

            # ---- Phase A2a: k,v,a projections + local scans + summaries ----
            with tc.tile_pool(name="psa", bufs=6, space="PSUM") as psa:
                for p in range(NPT):
                    ks = None
                    for wname, key in (("wk", "k"), ("wv", "v"), ("wa", "a")):
                        pts = [psa.tile([P, H], F32, tag="psa",
                                        name=f"ps_{key}{p}h{h}")
                               for h in range(2)]
                        wt = wpool.tile([P, D], F16, tag="w",
                                        name=f"w_{key}{p}")
                        nc.sync.dma_start(out=wt[:], in_=w_in[wname][p])
                        for k in range(NPT):
                            for h in range(2):
                                nc.tensor.matmul(
                                    pts[h][:],
                                    lhsT=wt[:, k * P:(k + 1) * P],
                                    rhs=xnT[k][:, h * H:(h + 1) * H],
                                    start=(k == 0), stop=(k == NPT - 1))
                        for h in range(2):
                            hs = slice(h * H, (h + 1) * H)
                            if key == "k":
                                if ks is None:
                                    ks = [scr.tile([P, H], F32, tag="ks",
                                                   name=f"ks{p}h{hh}")
                                          for hh in range(2)]
                                nc.vector.tensor_copy(ks[h][:], pts[h][:])
                            elif key == "v":
                                nc.vector.tensor_mul(
                                    kv_t[p][:, hs], pts[h][:], ks[h][:])
                            elif key == "a":
                                nc.scalar.activation(a_t[p][:, hs],
                                                     pts[h][:], AFT.Sigmoid)
                    nc.vector.tensor_tensor_scan(
                        sl_t[p][:], a_t[p][:], kv_t[p][:], 0.0,
                        op0=ALU.mult, op1=ALU.add)
                    nc.vector.tensor_tensor_scan(
                        ca_t[p][:], a_t[p][:], a_t[p][:], 1.0,
                        op0=ALU.mult, op1=ALU.bypass)
                    nc.vector.tensor_copy(summ[:, p:p + 1],
                                          ca_t[p][:, CHUNK - 1:CHUNK])
                    nc.vector.tensor_copy(summ[:, NPT + p:NPT + p + 1],
                                          sl_t[p][:, CHUNK - 1:CHUNK])

            # ---- summary exchange: single AllGather, triggered ASAP ----
            import os as _os
            _nocc = bool(int(_os.environ.get("NOCC", "0")))
            cc_in = dram.tile([P, SW], F32, name="cc_in")
            cc_out = dram.tile([NCORE, P, SW], F32, addr_space="Shared",
                               name="cc_out")
            nc.sync.dma_start(out=cc_in[:], in_=summ[:])
            if not _nocc:
                nc.gpsimd.collective_compute(
                    "AllGather", ALU.bypass,
                    replica_groups=[list(range(NCORE))],
                    ins=[cc_in[:]], outs=[cc_out[:]])

            # prefetch all output-projection weights while the gather flies
            woh = [[wopool.tile([P, H], F16, tag="woh", name=f"wo{h}k{k}")
                    for k in range(NPT)] for h in range(2)]
            for h in range(2):
                for k in range(NPT):
                    nc.sync.dma_start(out=woh[h][k][:], in_=w_in["wo"][h, k])

            # ---- Phase A2b: g,q projections; P = qg*sl, C = qg*ca ----
            with tc.tile_pool(name="psb", bufs=4, space="PSUM") as psb:
                for p in range(NPT):
                    gs = [scr.tile([P, H], F32, tag="gs", name=f"gs{p}h{hh}")
                          for hh in range(2)]
                    qg_p = scr.tile([P, CHUNK], F32, tag="qg", name=f"qg{p}")
                    for wname, key in (("wg", "g"), ("wq", "q")):
                        pts = [psb.tile([P, H], F32, tag="psb",
                                        name=f"ps_{key}{p}h{h}")
                               for h in range(2)]
                        wt = wpool.tile([P, D], F16, tag="w",
                                        name=f"w_{key}{p}")
                        nc.sync.dma_start(out=wt[:], in_=w_in[wname][p])
                        for k in range(NPT):
                            for h in range(2):
                                nc.tensor.matmul(
                                    pts[h][:],
                                    lhsT=wt[:, k * P:(k + 1) * P],
                                    rhs=xnT[k][:, h * H:(h + 1) * H],
                                    start=(k == 0), stop=(k == NPT - 1))
                        for h in range(2):
                            if key == "g":
                                nc.scalar.activation(gs[h][:], pts[h][:],
                                                     AFT.Silu)
                            else:
                                nc.vector.scalar_tensor_tensor(
                                    out=qg_p[:, h * H:(h + 1) * H],
                                    in0=pts[h][:], scalar=1.0 / YSC,
                                    in1=gs[h][:],
                                    op0=ALU.mult, op1=ALU.mult)
                    # P = qg * s_local (overwrites kv); C = qg * cumA
                    # (overwrites a)
                    nc.vector.tensor_mul(kv_t[p][:], qg_p[:], sl_t[p][:])
                    nc.vector.tensor_mul(a_t[p][:], qg_p[:], ca_t[p][:])

            # ---- gather consume + prefix combine ----
            gath = small.tile([P, NCORE * SW], F32)
            sin = small.tile([P, NPT], F32)
            if _nocc:
                nc.vector.memset(gath[:], 0.0)
            else:
                for c in range(NCORE):
                    nc.sync.dma_start(
                        out=gath[:, c * SW:(c + 1) * SW], in_=cc_out[c])

            def A_of(j):
                return gath[:, j * SW: j * SW + NPT]

            def s_of(j):
                return gath[:, j * SW + NPT: j * SW + SW]

            cand = small.tile([P, NPT, NCORE], F32, name="cand")
            nc.vector.memset(cand[:], 0.0)
            u = small.tile([P, NPT], F32, name="u")
            tmp = small.tile([P, NPT], F32, name="tmp")
            for g in range(GROUPS):
                base = g * CPG
                nc.vector.tensor_copy(u[:], s_of(base))
                nc.vector.tensor_copy(cand[:, :, base + 1], u[:])
                for jj in range(2, CPG):
                    nc.vector.tensor_mul(tmp[:], A_of(base + jj - 1), u[:])
                    nc.vector.tensor_add(u[:], tmp[:], s_of(base + jj - 1))
                    nc.vector.tensor_copy(cand[:, :, base + jj], u[:])
            masked = small.tile([P, NPT, NCORE], F32, name="masked")
            nc.vector.tensor_mul(masked[:], cand[:], maskt[:])
            nc.vector.tensor_reduce(sin[:], masked[:],
                                    axis=mybir.AxisListType.X, op=ALU.add)

            # ---- Phase B: apply carry, output projection ----
            y_t = [xny.tile([P, CHUNK], F16, tag="xny", name=f"y{p}")
                   for p in range(NPT)]
            for p in range(NPT):
                nc.vector.scalar_tensor_tensor(
                    out=y_t[p][:], in0=a_t[p][:], scalar=sin[:, p:p + 1],
                    in1=kv_t[p][:], op0=ALU.mult, op1=ALU.add)

            with tc.tile_pool(name="pso", bufs=4, space="PSUM") as pso:
                for h in range(2):
                    for m in range(NM):
                        po = pso.tile([P, H], F32, tag="pso",
                                      name=f"po{h}m{m}")
                        for k in range(NPT):
                            nc.tensor.matmul(
                                po[:], lhsT=y_t[k][:, m * P:(m + 1) * P],
                                rhs=woh[h][k][:],
                                start=(k == 0), stop=(k == NPT - 1))
                        ostg = scr.tile([P, H], F32, tag="ostg",
                                        name=f"ostg{h}m{m}", bufs=3)
                        nc.scalar.activation(ostg[:], po[:], AFT.Copy,
                                             scale=YSC)
                        nc.sync.dma_start(
                            out=out_t[m * P:(m + 1) * P, h * H:(h + 1) * H],
                            in_=ostg[:])

    nc.compile()
    return nc


def tc_ctx(nc):
    return tile.TileContext(nc)


def _get_nc():
    if "nc" not in _CACHE:
        _CACHE["nc"] = _build()
    return _CACHE["nc"]


def _blk_proj(w):
    # [din, dout] -> [p, r, (k, c)]: per dout-ptile slab, contiguous
    return np.ascontiguousarray(
        w.reshape(NPT, P, NPT, P).transpose(2, 1, 0, 3).reshape(NPT, P, D)
        .astype(np.float16))


def _blk_out(w):
    # [din, dout] -> [h, k, r, c]
    return np.ascontiguousarray(
        w.reshape(NPT, P, 2, H).transpose(2, 0, 1, 3).astype(np.float16))


def _make_in_maps(x, gamma, wq, wk, wv, wa, wg, wo):
    w_eff = {
        "wq": _blk_proj(gamma[:, None] * wq),
        "wk": _blk_proj(gamma[:, None] * wk),
        "wv": _blk_proj(gamma[:, None] * wv),
        "wa": _blk_proj(gamma[:, None] * wa),
        "wg": _blk_proj(gamma[:, None] * wg),
        "wo": _blk_out(wo),
    }
    in_maps = []
    for c in range(NCORE):
        b, ch = divmod(c, CPG)
        mask = np.zeros((P, NPT, NCORE), dtype=np.float32)
        mask[:, :, c] = 1.0
        in_maps.append({
            "x": np.ascontiguousarray(
                x[b, ch * CHUNK:(ch + 1) * CHUNK, :], dtype=np.float32),
            "mask": mask.reshape(P, NPT * NCORE),
            **w_eff,
        })
    return in_maps


def run_device(in_maps, trace=False, **kw):
    return run_bass_kernel_spmd(_get_nc(), in_maps, list(range(NCORE)),
                                trace=trace, **kw)


def _assemble(results):
    out = np.empty((B, S, D), dtype=np.float32)
    for c in range(NCORE):
        b, ch = divmod(c, CPG)
        out[b, ch * CHUNK:(ch + 1) * CHUNK, :] = results[c]["out"]
    return out


def kernel(x, gamma, wq, wk, wv, wa, wg, wo):
    in_maps = _make_in_maps(np.asarray(x), np.asarray(gamma), np.asarray(wq),
                            np.asarray(wk), np.asarray(wv), np.asarray(wa),
                            np.asarray(wg), np.asarray(wo))
    res = run_device(in_maps)
    return _assemble(res.results)
